# revision 52
# baseline (speedup 1.0000x reference)
"""Trainium2 Bass kernel for nn_CorrelationLayer (v3).

Reference computation (per sample, C=256, H=W=64, s=8):
    corr  = 0.5*(corr_branch(x0) + corr_branch(x1))        # [64, H, W]
    red   = relu(instance_norm(conv1x1(corr, w_red1)))     # b_red1 cancels in IN
    red   = conv3x3(red, w_red2) + b_red2                  # [256, H, W]
    new   = relu(conv1x1(concat(x0, red), w_adapt) + b_adapt)
    depth = instance_norm(x1)
where corr_branch(x) = l2norm_c(avgpool8(x)) . l2norm_c(x) (cosine maps).

Sharding: pure data parallel, 2 samples per core on 8 cores.

v3 design (vs v2):
  - Single-pass 1/sqrt via the Abs_reciprocal_sqrt ACT function (pinned
    table "abs_reciprocal_sqrt_and_small", which also has Square / Relu /
    Identity) instead of the Exp(-0.5*Ln) two-pass chain.
  - cs and cp matmuls stack BOTH inputs into one [128,512] PSUM tile via
    out-partition offsets, halving PSUM traffic and eviction count.
  - Per-chunk squares into small scratch tiles (saves 28KB SBUF) split
    DVE(x0) / ACT-Square-with-accum(x1, also yields the per-channel
    sum-of-squares for depth_feat's instance norm for free).
  - Per-chunk pooling stage 1 on DVE + one full-tile reduce on Pool
    (gpsimd), balancing the P1 critical path across three engines.
  - red1 / conv3x3 / adapt accumulate mt-pairs into [128,1024] dual-bank
    PSUM tiles drained by single merged evictions; conv bias b_red2 is
    folded into the adapt bias on the host (ba' = ba + Wa1 @ b2), so conv
    evictions are pure copies.
  - depth_feat on the Pool engine; stores ride the SP queue.
  - Tighter two-sample weave: s1's whole P1+P2 hides under s0's conv
    window, adapt units and stores woven between conv chunks.
"""

import sys

sys.path.insert(0, "/opt/trn_rl_repo")

import numpy as np
from contextlib import ExitStack

import concourse.bass as bass
import concourse.tile as tile
from concourse import bacc, mybir
from concourse.bass_utils import run_bass_kernel_spmd

AF = mybir.ActivationFunctionType
ALU = mybir.AluOpType
AX = mybir.AxisListType
F32 = mybir.dt.float32
BF16 = mybir.dt.bfloat16
F8 = mybir.dt.float8e4
DR = mybir.MatmulPerfMode.DoubleRow
SC = 16.0             # fp8 weight-residual scale

N_CORES = 8
B, C, H, W = 16, 256, 64, 64
HW = H * W            # 4096
S2 = 64               # corr_size**2
SPC = B // N_CORES    # samples per core = 2
NCH = 8               # 512-px chunks per image
CHUNK = HW // NCH     # 512
EPS_IN = 1e-5

_CACHE = {}


class _Bacc(bacc.Bacc):
    """Bacc with the ACT-table chooser pinned to one set covering every
    function this kernel uses (abs_reciprocal_sqrt / square / relu /
    identity / copy).  The default greedy chooser can alternate between
    partial sets and insert per-chunk table loads at 1283 ns each."""

    _ACT_SET = "abs_reciprocal_sqrt_and_small"

    def insert_act_table_loads(self):
        import concourse.mybir as _mb
        from concourse.hw_specs import get_activation_tables
        import bass_rust as _br
        has_activation = any(
            isinstance(i, _mb.InstActivation)
            for blk in self.main_func.blocks
            for i in blk.instructions
        )
        if not has_activation:
            return
        tables = [
            (name, funcs if name == self._ACT_SET else set())
            for name, funcs in get_activation_tables(self.m.arch).items()
        ]
        _br.insert_act_table_loads(self, tables)


def _build_program():
    nc = _Bacc("TRN2", target_bir_lowering=False, debug=False,
               num_devices=N_CORES)

    x0_d = nc.dram_tensor("x0", [SPC, C, HW], BF16, kind="ExternalInput").ap()
    x1_d = nc.dram_tensor("x1", [SPC, C, HW], BF16, kind="ExternalInput").ap()
    xd = (x0_d, x1_d)
    w1t_d = nc.dram_tensor("w1t", [S2, C], BF16, kind="ExternalInput").ap()
    w2f_d = nc.dram_tensor("w2f", [2, 128, 2, 9 * C], F8,
                           kind="ExternalInput").ap()
    wat_d = nc.dram_tensor("wat", [128, 4 * C], BF16, kind="ExternalInput").ap()
    ba2_d = nc.dram_tensor("ba2", [128, 2], F32, kind="ExternalInput").ap()
    nf_d = nc.dram_tensor("nf", [SPC, C, HW], BF16, kind="ExternalOutput").ap()
    df_d = nc.dram_tensor("df", [SPC, C, HW], BF16, kind="ExternalOutput").ap()

    with tile.TileContext(nc) as tc, ExitStack() as ctx:
        tcp = lambda **kw: ctx.enter_context(tc.tile_pool(**kw))
        p_w = tcp(name="weights", bufs=1)
        p_x = tcp(name="x", bufs=7)                  # [128,4096] bf16
        p_sq = tcp(name="sq", bufs=7)                # [128,512] bf16 scratch
        p_p1 = tcp(name="p1", bufs=4)                # [128,512] f32
        p_ipx = tcp(name="ipx", bufs=1)              # [128,4096] f32
        p_small = tcp(name="small", bufs=27)
        p_corr = tcp(name="corr", bufs=1)            # [64,4096] bf16
        p_tt = tcp(name="tt", bufs=4)                # [64,512] bf16
        p_rr = tcp(name="rr", bufs=1)                # [128,2,4096] bf16
        p_pad = tcp(name="pad", bufs=4)              # [128,2,4360] fp8
        p_r2 = tcp(name="r2", bufs=1)                # [128,2,4096] bf16
        p_nf = tcp(name="nf", bufs=1)                # [128,2,4096] bf16
        p_df = tcp(name="df", bufs=2)                # [128,2048] bf16

        ps_s = tcp(name="ps_s", bufs=2, space="PSUM")   # [128,512] cs/cp
        ps_b = tcp(name="ps_b", bufs=3, space="PSUM")   # [128,1024] r1/cv/ad

        # ---- constants & weights (once) ----
        ones = p_w.tile([128, 128], BF16)
        nc.vector.memset(ones[:], 1.0)
        eps_c = p_w.tile([128, 1], F32)
        nc.vector.memset(eps_c[:], EPS_IN)
        w1t_sb = p_w.tile([S2, C], BF16)
        w2f_sb = [p_w.tile([128, 2, 9 * C], F8, name=f"w2f_sb{v}")
                  for v in range(2)]
        wat_sb = p_w.tile([128, 4 * C], BF16)
        ba2_sb = p_w.tile([128, 2], F32)

        def rsqrt(out, in_, scale=1.0, eps=False):
            # out = 1/sqrt(scale*in + eps)
            nc.scalar.activation(out, in_, AF.Abs_reciprocal_sqrt,
                                 bias=(eps_c[:] if eps else 0.0), scale=scale)

        def load_weights():
            nc.sync.dma_start(w1t_sb[:], w1t_d[:])
            nc.sync.dma_start(ba2_sb[:], ba2_d[:])
            nc.sync.dma_start(wat_sb[:], wat_d[:])
            for v in range(2):
                nc.sync.dma_start(w2f_sb[v][:], w2f_d[v])

        st = [dict() for _ in range(SPC)]

        def load_x(s, t):
            # t=0 tiles on the SP queue, t=1 tiles on the Pool queue;
            # half-tile pieces, first halves of both inputs ahead of the
            # second halves so chunk-0 compute starts ~3.5us earlier
            d = st[s]
            d.setdefault('x', [[None, None], [None, None]])
            q = nc.sync if t == 0 else nc.gpsimd
            for i in range(2):
                d['x'][i][t] = p_x.tile([128, HW], BF16,
                                        name=f"x{i}_{s}_{t}", tag="x")
            for h in range(2):
                for i in range(2):
                    sl = slice(h * 2048, (h + 1) * 2048)
                    q.dma_start(d['x'][i][t][:, sl],
                                xd[i][s, t * 128:(t + 1) * 128, sl])

        def p1_head(s):
            d = st[s]
            d['sq'] = [[None, None], [None, None]]
            d['p1'] = [[None, None], [None, None]]
            d['ipx'] = p_ipx.tile([128, HW], F32, name=f"ipx_{s}", tag="ipx")
            d['ssp'] = [p_small.tile([128, NCH], F32, name=f"ssp_{s}_{t}",
                                     tag="small") for t in range(2)]
            for i in range(2):
                for t in range(2):
                    d['p1'][i][t] = p_p1.tile([128, CHUNK], F32,
                                              name=f"p1_{s}_{i}_{t}", tag="p1")

        def p1_sq_x0(s, ch):
            # x0 squares for chunk ch on DVE (bf16 2x) + p1 pieces
            d = st[s]
            sl = slice(ch * CHUNK, (ch + 1) * CHUNK)
            for t in range(2):
                sq = p_sq.tile([128, CHUNK], BF16, name=f"sq0_{s}_{t}_{ch}",
                               tag="sq")
                xt = d['x'][0][t]
                nc.vector.tensor_tensor(sq[:], xt[:, sl], xt[:, sl], ALU.mult)
                d['sq'][0][t] = sq
                nc.vector.tensor_reduce(
                    d['p1'][0][t][:, ch * 64:(ch + 1) * 64],
                    xt[:, sl].rearrange("p (g w) -> p g w", w=8),
                    AX.X, ALU.add)

        def p1_sq_x1(s, ch):
            # x1 squares on ACT (Square + accum -> per-channel ss partials)
            d = st[s]
            sl = slice(ch * CHUNK, (ch + 1) * CHUNK)
            for t in range(2):
                sq = p_sq.tile([128, CHUNK], BF16, name=f"sq1_{s}_{t}_{ch}",
                               tag="sq")
                nc.scalar.activation(sq[:], d['x'][1][t][:, sl], AF.Square,
                                     accum_out=d['ssp'][t][:, ch:ch + 1])
                d['sq'][1][t] = sq

        def p1_red_x1(s, ch):
            # x1 pooling stage-1 per chunk on DVE (small ops interleave with
            # the squares stream instead of head-of-line blocking it)
            d = st[s]
            sl = slice(ch * CHUNK, (ch + 1) * CHUNK)
            for t in range(2):
                nc.vector.tensor_reduce(
                    d['p1'][1][t][:, ch * 64:(ch + 1) * 64],
                    d['x'][1][t][:, sl].rearrange("p (g w) -> p g w", w=8),
                    AX.X, ALU.add)

        def p1_cs(s, ch):
            # stacked per-pixel sum-of-squares -> 0.5/sqrt on ACT
            d = st[s]
            csp = ps_s.tile([128, CHUNK], F32, name=f"cs_{s}_{ch}", tag="ps_s")
            for i in range(2):
                for t in range(2):
                    nc.tensor.matmul(csp[i * S2:(i + 1) * S2, :],
                                     ones[:, :S2], d['sq'][i][t][:],
                                     start=(t == 0), stop=(t == 1))
            rsqrt(d['ipx'][:, ch * CHUNK:(ch + 1) * CHUNK], csp[:], scale=4.0)

        def p1_tail(s):
            d = st[s]
            khat = []
            for i in range(2):
                kh_t, psq = [], []
                for t in range(2):
                    pk = p_small.tile([128, S2], F32,
                                      name=f"pooled_{s}_{i}_{t}", tag="small")
                    # p1 index = 64*a + 8*r + w  ->  sum over r
                    nc.vector.tensor_reduce(
                        pk[:],
                        d['p1'][i][t][:].rearrange("p (a r w) -> p a w r",
                                                   a=8, r=8),
                        AX.X, ALU.add)
                    kh_t.append(pk)
                    pq = p_small.tile([128, S2], BF16, name=f"psq_{s}_{i}_{t}",
                                      tag="small")
                    nc.vector.tensor_tensor(pq[:], pk[:], pk[:], ALU.mult)
                    psq.append(pq)
                nkp = ps_s.tile([128, S2], F32, name=f"nk_{s}_{i}", tag="ps_s")
                for t in range(2):
                    nc.tensor.matmul(nkp[:], ones[:], psq[t][:],
                                     start=(t == 0), stop=(t == 1))
                invk = p_small.tile([128, S2], F32, name=f"invk_{s}_{i}",
                                    tag="small")
                rsqrt(invk[:], nkp[:])
                kh = []
                for t in range(2):
                    k2 = p_small.tile([128, S2], BF16,
                                      name=f"khat_{s}_{i}_{t}", tag="small")
                    nc.vector.tensor_tensor(k2[:], kh_t[t][:], invk[:],
                                            ALU.mult)
                    kh.append(k2)
                khat.append(kh)
            d['khat'] = khat

            # depth-feat (instance norm of x1): var = E[x^2] - E[x]^2
            istd1, bneg1 = [], []
            for t in range(2):
                s1 = p_small.tile([128, 1], F32, name=f"s1_{s}_{t}",
                                  tag="small")
                nc.vector.tensor_reduce(s1[:], d['p1'][1][t][:], AX.X, ALU.add)
                ssc = p_small.tile([128, 1], F32, name=f"ssc_{s}_{t}",
                                   tag="small")
                nc.vector.tensor_reduce(ssc[:], d['ssp'][t][:], AX.X, ALU.add)
                mean = p_small.tile([128, 1], F32, name=f"mean1_{s}_{t}",
                                    tag="small")
                nc.vector.tensor_scalar(mean[:], s1[:], 1.0 / HW, None,
                                        ALU.mult)
                ex2 = p_small.tile([128, 1], F32, name=f"ex2_{s}_{t}",
                                   tag="small")
                nc.vector.tensor_scalar(ex2[:], ssc[:], 1.0 / HW, None,
                                        ALU.mult)
                msq = p_small.tile([128, 1], F32, name=f"msq_{s}_{t}",
                                   tag="small")
                nc.vector.tensor_tensor(msq[:], mean[:], mean[:], ALU.mult)
                var = p_small.tile([128, 1], F32, name=f"var1_{s}_{t}",
                                   tag="small")
                nc.vector.tensor_tensor(var[:], ex2[:], msq[:], ALU.subtract)
                ist = p_small.tile([128, 1], F32, name=f"istd1_{s}_{t}",
                                   tag="small")
                rsqrt(ist[:], var[:], eps=True)
                bn = p_small.tile([128, 1], F32, name=f"bneg1_{s}_{t}",
                                  tag="small")
                nc.vector.scalar_tensor_tensor(bn[:], mean[:], -1.0, ist[:],
                                               ALU.mult, ALU.mult)
                istd1.append(ist)
                bneg1.append(bn)
            d['istd1'] = istd1
            d['bneg1'] = bneg1

        def df_unit(s, t, h):
            # depth_feat half-tile on Pool; store on SP queue
            d = st[s]
            sl = slice(h * 2048, (h + 1) * 2048)
            dfc = p_df.tile([128, 2048], BF16, name=f"dfc_{s}_{t}_{h}",
                            tag="df")
            nc.gpsimd.tensor_scalar(dfc[:], d['x'][1][t][:, sl],
                                    d['istd1'][t][:], d['bneg1'][t][:],
                                    ALU.mult, ALU.add)
            nc.sync.dma_start(df_d[s, t * 128:(t + 1) * 128, sl], dfc[:])

        def p2_head(s):
            d = st[s]
            d['corr'] = p_corr.tile([S2, HW], BF16, name=f"corr_{s}",
                                    tag="corr")
            d['rr'] = p_rr.tile([128, 2, HW], BF16, name=f"rr_{s}", tag="rr")
            if s == 0:
                d['rsum'] = [p_small.tile([128, NCH], F32,
                                          name=f"rs_{s}_{mt}", tag="small")
                             for mt in range(2)]
                d['r2sum'] = [p_small.tile([128, NCH], F32,
                                           name=f"r2s_{s}_{mt}", tag="small")
                              for mt in range(2)]
            else:
                d['bnsr'] = [p_small.tile([128, NCH * 6], F32,
                                          name=f"bnsr_{s}_{mt}", tag="small")
                             for mt in range(2)]

        def p2_cp(s, ch):
            # stacked cosine-map matmuls + per-pixel-norm combine
            d = st[s]
            sl = slice(ch * CHUNK, (ch + 1) * CHUNK)
            cpp = ps_s.tile([128, CHUNK], F32, name=f"cp_{s}_{ch}", tag="ps_s")
            for i in range(2):
                for t in range(2):
                    nc.tensor.matmul(cpp[i * S2:(i + 1) * S2, :],
                                     d['khat'][i][t][:], d['x'][i][t][:, sl],
                                     start=(t == 0), stop=(t == 1))
            tts = [p_tt.tile([S2, CHUNK], BF16, name=f"tt_{s}_{ch}_{i}",
                             tag="tt") for i in range(2)]
            for i in range(2):
                nc.vector.scalar_tensor_tensor(
                    tts[i][:], cpp[i * S2:(i + 1) * S2, :], 1.0,
                    d['ipx'][i * S2:(i + 1) * S2, sl], ALU.mult, ALU.mult)
            nc.vector.tensor_tensor(d['corr'][:, sl], tts[0][:], tts[1][:],
                                    ALU.add)

        def p2_red1(s, ch):
            # red1 mt-pair into dual-bank PSUM.  s0 (prologue, DVE-bound):
            # stats via ACT accum on per-mt evictions + a Square pass.
            # s1 (hidden under the conv window, DVE idle): DVE bn_stats.
            d = st[s]
            sl = slice(ch * CHUNK, (ch + 1) * CHUNK)
            rbp = ps_b.tile([128, 1024], F32, name=f"r1_{s}_{ch}", tag="ps_b")
            for mt in range(2):
                nc.tensor.matmul(rbp[:, mt * 512:(mt + 1) * 512],
                                 w1t_sb[:, mt * 128:(mt + 1) * 128],
                                 d['corr'][:, sl], start=True, stop=True)
            if s == 0:
                for mt in range(2):
                    half = rbp[:, mt * 512:(mt + 1) * 512]
                    nc.scalar.activation(
                        d['rr'][:, mt, sl], half, AF.Identity,
                        accum_out=d['rsum'][mt][:, ch:ch + 1])
                    scr = p_sq.tile([128, CHUNK], BF16,
                                    name=f"rsq_{s}_{ch}_{mt}", tag="sq")
                    nc.scalar.activation(
                        scr[:], half, AF.Square,
                        accum_out=d['r2sum'][mt][:, ch:ch + 1])
            else:
                nc.scalar.activation(
                    d['rr'][:, :, sl],
                    rbp[:].rearrange("p (k c) -> p k c", k=2), AF.Identity)
                for mt in range(2):
                    nc.vector.bn_stats(d['bnsr'][mt][:, ch * 6:(ch + 1) * 6],
                                       rbp[:, mt * 512:(mt + 1) * 512])

        def p2_unit(s, ch):
            p2_cp(s, ch)
            if ch >= 2:
                p2_red1(s, ch - 2)

        def p2_drain(s):
            p2_red1(s, NCH - 2)
            p2_red1(s, NCH - 1)

        def p2_tail(s):
            # pads: fp8 grids at offset 1, row pitch 66; two scale variants
            # (v=0: red, v=1: red/SC for the w-residual DoubleRow planes)
            d = st[s]
            pads = [p_pad.tile([128, 2, 4360], F8, name=f"pad_{s}_{v}",
                               tag="pad") for v in range(2)]
            d['pad'] = pads
            for v in range(2):
                pv = pads[v][:, :, 1:1 + 4356].rearrange(
                    "p k (h w) -> p k h w", w=66)
                for kt in range(2):
                    for brd in (pv[:, kt, 0:1, :], pv[:, kt, 65:66, :],
                                pv[:, kt, 1:65, 0:1], pv[:, kt, 1:65, 65:66]):
                        nc.gpsimd.memset(brd, 0.0)
                nc.gpsimd.memset(pads[v][:, :, 0:1], 0.0)
                nc.gpsimd.memset(pads[v][:, :, 4357:4360], 0.0)
            for mt in range(2):
                mv = p_small.tile([128, 2], F32, name=f"mvr_{s}_{mt}",
                                  tag="small")
                if s == 0:
                    # mean/var from the ACT accum sums: var = E[r2] - E[r]^2
                    sm = p_small.tile([128, 2], F32, name=f"sm_{s}_{mt}",
                                      tag="small")
                    nc.vector.tensor_reduce(sm[:, 0:1], d['rsum'][mt][:],
                                            AX.X, ALU.add)
                    nc.vector.tensor_reduce(sm[:, 1:2], d['r2sum'][mt][:],
                                            AX.X, ALU.add)
                    nc.vector.tensor_scalar(mv[:], sm[:], 1.0 / HW, None,
                                            ALU.mult)
                    msq = p_small.tile([128, 1], F32, name=f"msqr_{s}_{mt}",
                                      tag="small")
                    nc.vector.tensor_tensor(msq[:], mv[:, 0:1], mv[:, 0:1],
                                            ALU.mult)
                    nc.vector.tensor_tensor(mv[:, 1:2], mv[:, 1:2], msq[:],
                                            ALU.subtract)
                else:
                    nc.vector.bn_aggr(mv[:], d['bnsr'][mt][:])
                ist = p_small.tile([128, 2], F32, name=f"istdr_{s}_{mt}",
                                   tag="small")
                rsqrt(ist[:, 0:1], mv[:, 1:2], eps=True)
                nc.vector.tensor_scalar(ist[:, 1:2], ist[:, 0:1], 1.0 / SC,
                                        None, ALU.mult)
                bn = p_small.tile([128, 2], F32, name=f"bnegr_{s}_{mt}",
                                  tag="small")
                nc.vector.scalar_tensor_tensor(bn[:, 0:1], mv[:, 0:1], -1.0,
                                               ist[:, 0:1], ALU.mult, ALU.mult)
                nc.vector.tensor_scalar(bn[:, 1:2], bn[:, 0:1], 1.0 / SC,
                                        None, ALU.mult)
                d.setdefault('ist', []).append(ist)
                d.setdefault('bnr', []).append(bn)

        def pad_band(s, b):
            # write pad rows 16b..16b+15 (all mt, both scale variants) so
            # conv groups can start before the full pad is materialized
            d = st[s]
            for mt in range(2):
                ist, bn = d['ist'][mt], d['bnr'][mt]
                pv0 = d['pad'][0][:, :, 1:1 + 4356].rearrange(
                    "p k (h w) -> p k h w", w=66)
                pv1 = d['pad'][1][:, :, 1:1 + 4356].rearrange(
                    "p k (h w) -> p k h w", w=66)
                nc.scalar.activation(
                    pv0[:, mt, 1 + 16 * b:17 + 16 * b, 1:65],
                    d['rr'][:, mt, 1024 * b:1024 * (b + 1)].rearrange(
                        "p (h w) -> p h w", w=64),
                    AF.Relu, bias=bn[:, 0:1], scale=ist[:, 0:1])
                # lo-scale band = hi band / SC, exact fp8 exponent shift
                nc.vector.tensor_scalar(
                    pv1[:, mt, 1 + 16 * b:17 + 16 * b, 1:65],
                    pv0[:, mt, 1 + 16 * b:17 + 16 * b, 1:65],
                    1.0 / SC, None, ALU.mult)

        NCG = 11                       # 6-row conv groups (last has 4 rows)

        def conv_group(s, g):
            # conv3x3 via fp8 DoubleRow: both kt planes per instruction,
            # w_hi + w_lo*SC residual planes; contiguous 6-row windows with
            # zero-pad junk columns, trimmed at eviction
            d = st[s]
            rows = 6 if g < NCG - 1 else 4
            ncols = rows * 66
            cvp = ps_b.tile([128, 1024], F32, name=f"cv_{s}_{g}", tag="ps_b")
            for mt in range(2):
                first = True
                for off in range(9):
                    dy, dx = off // 3, off % 3
                    o = (6 * g + dy) * 66 + dx
                    for v in range(2):
                        nc.tensor.matmul(
                            cvp[:, mt * 512:mt * 512 + ncols],
                            w2f_sb[v][:, :, off * C + mt * 128:
                                      off * C + mt * 128 + 128],
                            d['pad'][v][:, :, o:o + ncols],
                            start=first, stop=(off == 8 and v == 1),
                            perf_mode=DR)
                        first = False
                dst = d['r2'][:, mt, 6 * g * 64:(6 * g + rows) * 64
                              ].rearrange("p (r c) -> p r c", c=64)
                srcv = cvp[:, mt * 512:mt * 512 + ncols].rearrange(
                    "p (r c) -> p r c", c=66)[:, :, 1:65]
                if s == 0:
                    # window 1 is ACT-bound; drain s0's conv PSUM on DVE
                    nc.vector.tensor_scalar(dst, srcv, 1.0, None, ALU.mult)
                else:
                    nc.scalar.activation(dst, srcv, AF.Identity)

        def p4_head(s):
            d = st[s]
            d['r2'] = p_r2.tile([128, 2, HW], BF16, name=f"r2_{s}", tag="r2")
            d['nf'] = p_nf.tile([128, 2, HW], BF16, name=f"nf_{s}", tag="nf")

        def p4_unit(s, mt, g):
            # adapt 1x1 (K=512) cc-pair into dual-bank PSUM; relu evict
            d = st[s]
            adp = ps_b.tile([128, 1024], F32, name=f"ad_{s}_{mt}_{g}",
                            tag="ps_b")
            for cc in range(2):
                gsl = slice(g * 1024 + cc * 512, g * 1024 + (cc + 1) * 512)
                for kt in range(4):
                    lhs = wat_sb[:, kt * C + mt * 128:kt * C + mt * 128 + 128]
                    rhs = (d['x'][0][kt][:, gsl] if kt < 2
                           else d['r2'][:, kt - 2, gsl])
                    nc.tensor.matmul(adp[:, cc * 512:(cc + 1) * 512], lhs, rhs,
                                     start=(kt == 0), stop=(kt == 3))
            # bias+relu eviction on DVE: (x + ba') max 0
            nc.vector.tensor_scalar(d['nf'][:, mt, g * 1024:(g + 1) * 1024],
                                    adp[:], ba2_sb[:, mt:mt + 1], 0.0,
                                    ALU.add, ALU.max)

        def nf_store(s, mt, h):
            d = st[s]
            sl = slice(h * 2048, (h + 1) * 2048)
            nc.sync.dma_start(nf_d[s, mt * 128:(mt + 1) * 128, sl],
                              d['nf'][:, mt, sl])

        def prologue(s):
            p1_head(s)
            for ch in range(NCH):
                p1_sq_x0(s, ch)
                p1_sq_x1(s, ch)
                p1_cs(s, ch)
                p1_red_x1(s, ch)
            p1_tail(s)
            p2_head(s)
            for ch in range(NCH):
                p2_unit(s, ch)
                if ch == 3:
                    df_unit(s, 0, 0)
                    df_unit(s, 0, 1)
                elif ch == 5:
                    df_unit(s, 1, 0)
                    df_unit(s, 1, 1)
            p2_drain(s)
            p2_tail(s)

        # ===== schedule =====
        load_x(0, 0)
        load_weights()
        load_x(0, 1)
        load_x(1, 0)
        prologue(0)
        load_x(1, 1)
        p4_head(0)

        # --- s0 conv window: weave s1's P1+P2 between conv/adapt units ---
        s1_work = []
        for ch in range(NCH):
            s1_work.append(lambda ch=ch: (p1_sq_x0(1, ch), p1_sq_x1(1, ch)))
            s1_work.append(lambda ch=ch: (p1_cs(1, ch), p1_red_x1(1, ch)))
        s1_work.append(lambda: p1_tail(1))
        s1_work.append(lambda: p2_head(1))
        for ch in range(NCH):
            s1_work.append(lambda ch=ch: p2_unit(1, ch))
            if ch == 3:
                s1_work.append(lambda: (df_unit(1, 0, 0), df_unit(1, 0, 1)))
            elif ch == 5:
                s1_work.append(lambda: (df_unit(1, 1, 0), df_unit(1, 1, 1)))
        s1_work.append(lambda: p2_drain(1))
        s1_work.append(lambda: p2_tail(1))

        def fire(queue, n):
            for _ in range(n):
                if queue:
                    queue.pop(0)()

        p1_head(1)
        pad_sched = {0: 0, 2: 1, 5: 2, 7: 3}
        for g in range(NCG):
            if g in pad_sched:
                pad_band(0, pad_sched[g])
            fire(s1_work, 3)
            conv_group(0, g)
        for u in range(8):
            mt, g = u % 2, u // 2
            p4_unit(0, mt, g)
            fire(s1_work, 1)
            if u == 5:
                nf_store(0, 0, 0)
                nf_store(0, 1, 0)
        nf_store(0, 0, 1)
        nf_store(0, 1, 1)
        fire(s1_work, len(s1_work))
        p4_head(1)

        # --- s1 conv window: weave s1's adapt + stores ---
        adapt_after = {3: 0, 6: 1, 8: 2}
        for g in range(NCG):
            if g in pad_sched:
                pad_band(1, pad_sched[g])
            conv_group(1, g)
            if g in adapt_after:
                ga = adapt_after[g]
                p4_unit(1, 0, ga)
                p4_unit(1, 1, ga)
            if g == 9:
                nf_store(1, 0, 0)
                nf_store(1, 1, 0)
        p4_unit(1, 0, 3)
        p4_unit(1, 1, 3)
        nf_store(1, 0, 1)
        nf_store(1, 1, 1)

    nc.compile()
    return nc


def _get_program():
    if "nc" not in _CACHE:
        _CACHE["nc"] = _build_program()
    return _CACHE["nc"]


def _bf16():
    import ml_dtypes
    return ml_dtypes.bfloat16


def _prep_weights(w_red1, w_red2, w_adapt, b_red2, b_adapt):
    import ml_dtypes
    bf = _bf16()
    f8 = ml_dtypes.float8_e4m3
    w1t = np.ascontiguousarray(w_red1[:, :, 0, 0].T).astype(bf)       # [64,256]
    # conv weights, fp8 hi + scaled residual: [v, p(ci in kt), kt, off*C+co]
    w2r = (w_red2.transpose(2, 3, 1, 0).reshape(9, 2, 128, C)
           .transpose(2, 1, 0, 3))                                    # p,kt,off,co
    whi = w2r.astype(f8)
    wlo = ((w2r - whi.astype(np.float32)) * SC).astype(f8)
    w2f = np.ascontiguousarray(
        np.stack([whi, wlo]).reshape(2, 128, 2, 9 * C))
    wat = np.ascontiguousarray(
        w_adapt[:, :, 0, 0].T.reshape(4, 128, C).transpose(1, 0, 2)
        .reshape(128, 4 * C)).astype(bf)
    # fold conv bias through the adapt conv: ba' = ba + Wa[:, C:2C] @ b2
    ba2 = b_adapt + w_adapt[:, C:2 * C, 0, 0] @ b_red2
    ba2 = np.ascontiguousarray(ba2.reshape(2, 128).T).astype(np.float32)
    return w1t, w2f, wat, ba2


def make_in_maps(x0, x1, w_red1, b_red1, w_red2, b_red2, w_adapt, b_adapt):
    bf = _bf16()
    w1t, w2f, wat, ba2 = _prep_weights(
        np.asarray(w_red1, np.float32), np.asarray(w_red2, np.float32),
        np.asarray(w_adapt, np.float32), np.asarray(b_red2, np.float32),
        np.asarray(b_adapt, np.float32))
    x0 = np.asarray(x0, np.float32).reshape(B, C, HW).astype(bf)
    x1 = np.asarray(x1, np.float32).reshape(B, C, HW).astype(bf)
    in_maps = []
    for i in range(N_CORES):
        sl = slice(i * SPC, (i + 1) * SPC)
        in_maps.append({
            "x0": np.ascontiguousarray(x0[sl]),
            "x1": np.ascontiguousarray(x1[sl]),
            "w1t": w1t, "w2f": w2f, "wat": wat, "ba2": ba2,
        })
    return in_maps


def kernel(x0, x1, w_red1, b_red1, w_red2, b_red2, w_adapt, b_adapt):
    nc = _get_program()
    in_maps = make_in_maps(x0, x1, w_red1, b_red1, w_red2, b_red2,
                           w_adapt, b_adapt)
    res = run_bass_kernel_spmd(nc, in_maps, list(range(N_CORES)))
    nf = np.concatenate([np.asarray(res.results[i]["nf"])
                         for i in range(N_CORES)], axis=0)
    df = np.concatenate([np.asarray(res.results[i]["df"])
                         for i in range(N_CORES)], axis=0)
    return (nf.astype(np.float32).reshape(B, C, H, W),
            df.astype(np.float32).reshape(B, C, H, W))
